# revision 3
# baseline (speedup 1.0000x reference)
"""Trainium2 Bass kernel for nn_ActorHead (GNN edge-MLP with pairwise mean).

Reference computation:
    sel_r/sel_s = edge_index[0/1][edge_type_idx]            # [EA]
    x = concat(h[:, sel_r], h[:, sel_s], edge_attr[:, edge_type_idx])  # [B, EA, 272]
    y = relu(x @ W1 + b1) @ W2 + b2                          # [B, EA, 2]
    out = y.reshape(B, EA//2, 2, 2).mean(axis=2)             # [B, EA//2, 2]

Strategy (8 NeuronCores, SPMD):
  - Edges are shared across the B=4 batches, so the node-feature table is
    packed host-side as htab[n] = [h0[n] | h1[n] | h2[n] | h3[n]] (512 bf16 =
    1024B rows): one gathered row serves all 4 batches (4x fewer descriptors).
  - Edge dim sharded across cores. On-device gather uses the dma_gather custom
    instruction (plain mode, 4 SWDGE queues round-robin for latency pipelining)
    with int16 indices relative to one of 4 windows of <=32768 table rows.
    Edges are grouped host-side by (r-window, s-window); group order is an
    arbitrary permutation which the host inverts afterwards, and the final
    pairwise mean (+b2) is applied on the host.
  - MLP on device in bf16: PE transposes gathered tiles to [feat, edge] layout,
    then y1 = relu(W1r^T hrT + W1s^T hsT + W1e^T eaT + b1), y2 = W2^T y1,
    batched over 4 batches per matmul (N=512).
"""

import numpy as np
import ml_dtypes

B, N, E, EA = 4, 100000, 160000, 80000
HID, ED = 128, 16
NCORES = 8
WSZ = 32768                      # window size (int16-addressable rows)
NW = (N + WSZ - 1) // WSZ        # 4 windows (3 full + 1696 stub)
MAX_OP_IDX = 2048                # max indices per dma_gather op
TMODE_FRAC = 0.5                 # fraction of rows gathered in transpose mode (queue 0)

_cache = {}


def _wrap_idx(rel):
    """int16 indices -> wrapped SBUF layout [128, len//16] (i -> [i%16, i//16],
    replicated down the 8 16-partition core groups)."""
    n = len(rel)
    assert n % 16 == 0
    w = rel.reshape(n // 16, 16).T.astype(np.int16)   # [16, n//16]
    return np.tile(w, (8, 1))                          # [128, n//16]


def _prepare(h, edge_index, edge_attr, edge_type_idx, W1, b1, W2, b2):
    bf16 = ml_dtypes.bfloat16
    sel = np.asarray(edge_index)[:, np.asarray(edge_type_idx)]     # [2, EA]
    sel_r = sel[0].astype(np.int64)
    sel_s = sel[1].astype(np.int64)

    wr = sel_r // WSZ
    ws = sel_s // WSZ
    gid = wr * NW + ws                                             # [EA]

    # per-group global edge lists, split evenly across cores
    group_edges = [np.nonzero(gid == g)[0] for g in range(NW * NW)]
    # uniform per-(core,group) padded sizes (same compiled graph on all cores)
    S = []
    for g in range(NW * NW):
        per_core = -(-len(group_edges[g]) // NCORES)               # ceil
        S.append(-(-max(per_core, 0) // 128) * 128 if per_core else 0)
    NPAD = int(sum(S))

    ea_sel = np.asarray(edge_attr)[:, np.asarray(edge_type_idx), :]  # [B, EA, ED]

    cores = []
    for c in range(NCORES):
        slot_edges = np.full(NPAD, -1, dtype=np.int64)
        idx_r = np.zeros(NPAD, dtype=np.int16)
        idx_s = np.zeros(NPAD, dtype=np.int16)
        off = 0
        for g in range(NW * NW):
            ge = group_edges[g]
            lo = (len(ge) * c) // NCORES
            hi = (len(ge) * (c + 1)) // NCORES
            part = ge[lo:hi]
            n = len(part)
            assert n <= S[g]
            slot_edges[off:off + n] = part
            idx_r[off:off + n] = (sel_r[part] - (sel_r[part] // WSZ) * WSZ).astype(np.int16)
            idx_s[off:off + n] = (sel_s[part] - (sel_s[part] // WSZ) * WSZ).astype(np.int16)
            off += S[g]

        # eat layout: [16 feat, tile, batch, 128] flattened to [16, NPAD*4]
        ea_pad = np.zeros((B, NPAD, ED), dtype=np.float32)
        valid = slot_edges >= 0
        ea_pad[:, valid, :] = ea_sel[:, slot_edges[valid], :]
        # [B, NPAD, ED] -> [ED, NPAD//128, B, 128]
        eat = ea_pad.reshape(B, NPAD // 128, 128, ED).transpose(3, 1, 0, 2)
        eat = np.ascontiguousarray(eat.reshape(ED, NPAD * B)).astype(bf16)

        idx_all = np.concatenate([_wrap_idx(idx_r), _wrap_idx(idx_s)], axis=1)
        cores.append({
            "slot_edges": slot_edges,
            "idx": idx_all,           # [128, 2*NPAD//16] int16
            "eat": eat,               # [16, NPAD*4] bf16
        })

    h_np = np.asarray(h, dtype=np.float32)
    htab = np.ascontiguousarray(h_np.transpose(1, 0, 2).reshape(N, B * HID)).astype(bf16)

    W1_np = np.asarray(W1, dtype=np.float32)
    wts = {
        "w1r": np.ascontiguousarray(W1_np[:HID]).astype(bf16),            # [128,128]
        "w1s": np.ascontiguousarray(W1_np[HID:2 * HID]).astype(bf16),     # [128,128]
        "w1e": np.ascontiguousarray(W1_np[2 * HID:]).astype(bf16),        # [16,128]
        "w2": np.ascontiguousarray(np.asarray(W2, dtype=np.float32)).astype(bf16),  # [128,2]
        "b1": np.asarray(b1, dtype=np.float32).reshape(HID, 1).copy(),
        "ident": np.eye(128, dtype=np.float32).astype(bf16),
    }
    meta = {"S": S, "NPAD": NPAD, "wr": None}
    return htab, wts, cores, meta, sel_r, sel_s


def _build(S, NPAD):
    import concourse.bass as bass
    import concourse.mybir as mybir
    from concourse import bacc
    from concourse.tile import TileContext

    bf = mybir.dt.bfloat16
    f32 = mybir.dt.float32

    nc = bacc.Bacc("TRN2", target_bir_lowering=False, debug=False,
                   num_devices=NCORES, num_swdge_queues=4)

    htab = nc.dram_tensor("htab", [N, B * HID], bf, kind="ExternalInput").ap()
    idx_ext = nc.dram_tensor("idx", [128, 2 * NPAD // 16], mybir.dt.int16,
                             kind="ExternalInput").ap()
    eat_ext = nc.dram_tensor("eat", [ED, NPAD * B], bf, kind="ExternalInput").ap()
    w1r_ext = nc.dram_tensor("w1r", [HID, HID], bf, kind="ExternalInput").ap()
    w1s_ext = nc.dram_tensor("w1s", [HID, HID], bf, kind="ExternalInput").ap()
    w1e_ext = nc.dram_tensor("w1e", [ED, HID], bf, kind="ExternalInput").ap()
    w2_ext = nc.dram_tensor("w2", [HID, 2], bf, kind="ExternalInput").ap()
    b1_ext = nc.dram_tensor("b1", [HID, 1], f32, kind="ExternalInput").ap()
    id_ext = nc.dram_tensor("ident", [128, 128], bf, kind="ExternalInput").ap()
    out_ext = nc.dram_tensor("out", [2, B, NPAD], f32, kind="ExternalOutput").ap()

    max_sg = max(S)
    RELU = mybir.ActivationFunctionType.Relu

    with TileContext(nc) as tc:
        with (
            tc.tile_pool(name="const", bufs=1) as cp,
            tc.tile_pool(name="gr", bufs=3) as grp,
            tc.tile_pool(name="gs", bufs=3) as gsp,
            tc.tile_pool(name="eap", bufs=2) as eap,
            tc.tile_pool(name="tsb", bufs=3) as tsb,
            tc.tile_pool(name="y1s", bufs=2) as y1sp,
            tc.tile_pool(name="y2s", bufs=2) as y2sp,
            tc.tile_pool(name="ptp", bufs=2, space="PSUM") as ptp,
            tc.tile_pool(name="y1p", bufs=2, space="PSUM") as y1pp,
            tc.tile_pool(name="y2p", bufs=2, space="PSUM") as y2pp,
        ):
            w1r = cp.tile([HID, HID], bf)
            nc.sync.dma_start(out=w1r[:], in_=w1r_ext[:])
            w1s = cp.tile([HID, HID], bf)
            nc.sync.dma_start(out=w1s[:], in_=w1s_ext[:])
            w1e = cp.tile([ED, HID], bf)
            nc.sync.dma_start(out=w1e[:], in_=w1e_ext[:])
            w2 = cp.tile([HID, 2], bf)
            nc.sync.dma_start(out=w2[:], in_=w2_ext[:])
            b1 = cp.tile([HID, 1], f32)
            nc.sync.dma_start(out=b1[:], in_=b1_ext[:])
            ident = cp.tile([128, 128], bf)
            nc.sync.dma_start(out=ident[:], in_=id_ext[:])
            idx_sb = cp.tile([128, 2 * NPAD // 16], mybir.dt.int16)
            nc.sync.dma_start(out=idx_sb[:], in_=idx_ext[:])

            # assign transpose-mode to a prefix of groups covering ~TMODE_FRAC rows
            total_rows = sum(S)
            tmode_groups = set()
            acc = 0
            for g in sorted(range(NW * NW), key=lambda g: -S[g]):
                if acc < TMODE_FRAC * total_rows and S[g] > 0:
                    tmode_groups.add(g)
                    acc += S[g]

            qc = 0
            goff = 0
            for g in range(NW * NW):
                sg = S[g]
                if sg == 0:
                    continue
                wrw = g // NW
                wsw = g % NW
                nt = sg // 128
                tmode = g in tmode_groups

                # gather: list of (tile, chunk0, nchunks) per stream
                ops = {"r": [], "s": []}
                for (sk, win, stream_off, pool) in (
                    ("r", wrw, 0, grp), ("s", wsw, NPAD // 16, gsp),
                ):
                    wlo = win * WSZ
                    whi = min(wlo + WSZ, N)
                    c0 = 0
                    while c0 < nt:
                        cn = min(nt - c0, MAX_OP_IDX // 128)
                        ni = cn * 128
                        icol = stream_off + (goff + c0 * 128) // 16
                        if tmode:
                            dst = pool.tile([128, B, ni], bf, tag=f"ht{sk}")
                            out_ap = dst[:]
                        else:
                            dst = pool.tile([128, cn, B * HID], bf, tag=f"hp{sk}")
                            out_ap = dst[:]
                        nc.gpsimd.dma_gather(
                            out_ap=out_ap,
                            in_ap=htab[wlo:whi],
                            idxs_ap=idx_sb[:, icol:icol + ni // 16],
                            num_idxs=ni,
                            num_idxs_reg=ni,
                            elem_size=B * HID,
                            transpose=tmode,
                            single_packet=False,
                            queue_num=0 if tmode else 1 + qc % 3,
                        )
                        if not tmode:
                            qc += 1
                        ops[sk].append((dst, c0, cn))
                        c0 += cn

                ea_g = eap.tile([ED, B * max_sg], bf, tag="ea")
                nc.sync.dma_start(out=ea_g[:, :B * sg],
                                  in_=eat_ext[:, B * goff:B * (goff + sg)])

                def _op_slice(oplist, t):
                    for (dst, c0, cn) in oplist:
                        if c0 <= t < c0 + cn:
                            return dst, t - c0
                    raise AssertionError

                for t in range(nt):
                    dr, lr = _op_slice(ops["r"], t)
                    ds, ls = _op_slice(ops["s"], t)
                    if tmode:
                        rhs_r = dr[:, :, lr * 128:(lr + 1) * 128]
                        rhs_s = ds[:, :, ls * 128:(ls + 1) * 128]
                    else:
                        ptr = ptp.tile([128, B * HID], bf, tag="ptr", space="PSUM")
                        pts = ptp.tile([128, B * HID], bf, tag="pts", space="PSUM")
                        for b in range(B):
                            nc.tensor.transpose(
                                out=ptr[:, b * HID:(b + 1) * HID],
                                in_=dr[:, lr, b * HID:(b + 1) * HID],
                                identity=ident[:],
                            )
                        for b in range(B):
                            nc.tensor.transpose(
                                out=pts[:, b * HID:(b + 1) * HID],
                                in_=ds[:, ls, b * HID:(b + 1) * HID],
                                identity=ident[:],
                            )
                        hrT = tsb.tile([128, B * HID], bf, tag="hrT")
                        nc.vector.tensor_copy(out=hrT[:], in_=ptr[:])
                        hsT = tsb.tile([128, B * HID], bf, tag="hsT")
                        nc.vector.tensor_copy(out=hsT[:], in_=pts[:])
                        rhs_r = hrT[:]
                        rhs_s = hsT[:]

                    y1 = y1pp.tile([128, B * HID], f32, tag="y1", space="PSUM")
                    nc.tensor.matmul(out=y1[:], lhsT=w1r[:], rhs=rhs_r,
                                     start=True, stop=False)
                    nc.tensor.matmul(out=y1[:], lhsT=w1s[:], rhs=rhs_s,
                                     start=False, stop=False)
                    nc.tensor.matmul(out=y1[:], lhsT=w1e[:],
                                     rhs=ea_g[:, t * B * HID:(t + 1) * B * HID],
                                     start=False, stop=True)

                    y1sb = y1sp.tile([128, B * HID], bf, tag="y1sb")
                    nc.scalar.activation(out=y1sb[:], in_=y1[:], func=RELU,
                                         bias=b1[:])

                    y2 = y2pp.tile([2, B * HID], f32, tag="y2", space="PSUM")
                    nc.tensor.matmul(out=y2[:], lhsT=w2[:], rhs=y1sb[:],
                                     start=True, stop=True)
                    y2sb = y2sp.tile([2, B * HID], f32, tag="y2sb")
                    nc.vector.tensor_copy(out=y2sb[:], in_=y2[:])
                    nc.sync.dma_start(
                        out=out_ext[:, :, goff + t * 128:goff + (t + 1) * 128],
                        in_=y2sb[:].rearrange("p (b e) -> p b e", b=B),
                    )
                goff += sg
    nc.compile()
    return nc


def _run(inputs, trace=False):
    from concourse.bass_utils import run_bass_kernel_spmd

    htab, wts, cores, meta, sel_r, sel_s = _prepare(**inputs)
    key = tuple(meta["S"])
    if key not in _cache:
        _cache[key] = _build(meta["S"], meta["NPAD"])
    nc = _cache[key]

    in_maps = []
    for c in range(NCORES):
        m = {"htab": htab, "eat": cores[c]["eat"], "idx": cores[c]["idx"]}
        m.update({k: wts[k] for k in ("w1r", "w1s", "w1e", "w2", "b1", "ident")})
        in_maps.append(m)

    res = run_bass_kernel_spmd(nc, in_maps, core_ids=list(range(NCORES)),
                               trace=trace)

    # unshard: out[core] is [2, B, NPAD] in permuted slot order
    y2 = np.zeros((B, EA, 2), dtype=np.float32)
    for c in range(NCORES):
        o = res.results[c]["out"]                      # [2, B, NPAD]
        se = cores[c]["slot_edges"]
        valid = se >= 0
        y2[:, se[valid], :] = o[:, :, valid].transpose(1, 2, 0)

    b2 = np.asarray(inputs["b2"], dtype=np.float32)
    out = 0.5 * (y2[:, 0::2, :] + y2[:, 1::2, :]) + b2[None, None, :]
    return out.astype(np.float32), res.exec_time_ns


def kernel(**inputs):
    out, _ = _run(inputs, trace=False)
    return out
